# revision 12
# baseline (speedup 1.0000x reference)
"""Distance-aware comb-pilot interpolator for Trainium2 (8 NeuronCores).

Math: out[b, 8k+r, c] = alpha[r]*H[b,k,c] + gamma[r]*H[b,k+1,c], pilots on the
comb loc[k] = 8k (k = 0..511), Nfft = 4096.  Two identities cut the work:

  alpha[r] + gamma[r] = 1  (up to the reference's 1e-12 eps)
      -> out_r = H_k + gamma[r]*D_k  with  D = H[k+1] - H[k]
  alpha[8-r] = gamma[r]  (weight symmetry of the exp-decay kernel)
      -> out_{8-r} = H_{k+1} - gamma[r]*D_k   (reuses the same product)

So per batch-tile only 5 tensor_scalar muls (r = 0..4, on ACT mostly) and 8
dense tensor add/sub ops (DVE 2x fp16 mode) produce all 4096 subcarriers.

The device computes fp16 end-to-end and writes the output in r-major order
out_dev[b, r, k, c] so every DVE op touches only unit-stride APs (strided
dst was measured to drop DVE to 1x mode); the host de-interleaves with a
numpy transpose and upcasts to fp32 (rel err ~8e-4, gate is 2e-2).

HBM traffic per core: 1.05 MB in + 8.39 MB out ~= 26.4 us at 358 GB/s.
Engine busy: DVE ~24 us, ACT ~20 us, GPSIMD ~13 us (loads h2/h3, D-subs for
tiles 1-3, the 16-col last-block tail) -- memory-bound as intended.

The last 8 subcarriers (i = 4088..4095) interpolate between pilot 511 at
4088 and a virtual pilot hN = 1.875*H[511] - 0.875*H[510] at 4095 (gap 7,
not 8): per-r coefficients on H[510]/H[511], three tiny broadcast GPSIMD
ops per tile writing o[:, r, 511, :].
"""

import sys

import numpy as np

for _p in ("/opt/trn_rl_repo", "/root/.axon_site/_ro/trn_rl_repo"):
    if _p not in sys.path:
        sys.path.append(_p)

import concourse.bass as bass
import concourse.tile as tile
from concourse import bacc, mybir
from concourse.bass_utils import run_bass_kernel_spmd

N_CORES = 8
B, NP, NFFT, SPACING = 4096, 512, 4096, 8
B_LOC = B // N_CORES  # batch rows per core
NSEG = NP - 1  # regular 8-wide segments (k = 0..510)
P = 128  # SBUF partitions
N_BT = B_LOC // P  # 128-batch tiles per core
RW = 2 * NP  # o-tile columns per r-phase (512 k-slots x 2)

_PROGRAM = None


def _build_program():
    """One Bass program, identical on all cores (pure data parallel)."""
    nc = bacc.Bacc("TRN2", target_bir_lowering=False, debug=False)
    f16 = mybir.dt.float16
    f32 = mybir.dt.float32
    ls = nc.dram_tensor("ls", [B_LOC, NP * 2], f16, kind="ExternalInput").ap()
    cg = nc.dram_tensor("cg", [P, 8], f32, kind="ExternalInput").ap()
    cl = nc.dram_tensor("cl", [P, 32], f16, kind="ExternalInput").ap()
    out = nc.dram_tensor("out", [B_LOC, NFFT * 2], f16, kind="ExternalOutput").ap()

    # Mul engine per (tile, r): 'V' entries run on DVE (tensor_scalar, 4x
    # mode, no cross-engine latency -- used at the ramp (t0) and the drain
    # (t3 r4)); everything else on ACT.  Adds are issued right after the mul
    # they consume; the r-major store groups are ordered by completion.
    DVE_MULS = {(0, 0), (0, 4), (1, 4), (2, 4), (3, 4)}
    ADD_ORDER = [0, 1, 7, 2, 6, 3, 5, 4]  # add_r after mul_min(r,8-r)
    SGROUPS = {
        0: [(0, 1), (1, 2), (6, 8), (2, 4), (4, 6)],
        1: [(0, 2), (6, 8), (2, 4), (4, 6)],
        2: [(0, 2), (6, 8), (2, 4), (4, 6)],
        3: [(0, 1), (1, 2), (7, 8), (2, 3), (6, 7), (3, 4), (5, 6), (4, 5)],
    }
    # t3's trailing 4 store configs ride ACT's ring (they sit after the last
    # ACT mul in its queue, so their sem-waits block nothing), keeping the
    # final drain to one 256 KB chunk.
    T3_ACT = {(6, 7), (3, 4), (5, 6), (4, 5)}

    with tile.TileContext(nc) as tc:
        with (
            tc.tile_pool(name="cpool", bufs=1) as cpool,
            tc.tile_pool(name="hpool", bufs=4) as hpool,
            tc.tile_pool(name="dpool", bufs=4) as dpool,
            tc.tile_pool(name="tpool", bufs=10) as tpool,
            tc.tile_pool(name="opool", bufs=4) as opool,
            tc.tile_pool(name="lpool", bufs=8) as lpool,
        ):
            # Loads: h0 first on ACT's HWDGE ring (fast first byte, gates the
            # whole pipeline), the tiny gamma tile right after; ctl + h1 on
            # SP's HWDGE (idle until the first store); h2/h3 via gpsimd SWDGE.
            hs = [hpool.tile([P, NP * 2], f16, name=f"h{t}", tag="h") for t in range(N_BT)]
            ct = cpool.tile([P, 8], f32)
            nc.sync.dma_start(ct[:], cg)
            nc.scalar.dma_start(hs[0][:], ls[0:P, :])
            ctl = cpool.tile([P, 32], f16)
            nc.sync.dma_start(ctl[:], cl)
            nc.sync.dma_start(hs[1][:], ls[P : 2 * P, :])
            # h2/h3 queue behind h0 on ACT's ring: h0 keeps HBM priority
            # during the ramp, Q7 stays free for the D subs.
            nc.scalar.dma_start(hs[2][:], ls[2 * P : 3 * P, :])
            nc.scalar.dma_start(hs[3][:], ls[3 * P : 4 * P, :])

            a_last = ctl[:, 0:16].rearrange("p (r c) -> p r c", c=2)
            c_last = ctl[:, 16:32].rearrange("p (r c) -> p r c", c=2)

            os_ = [opool.tile([P, NFFT * 2], f16, name=f"o{t}", tag="o") for t in range(N_BT)]
            orvs = [o[:].rearrange("p (r k) -> p r k", r=SPACING) for o in os_]
            ds = [dpool.tile([P, 2 * NSEG], f16, name=f"d{t}", tag="d") for t in range(N_BT)]

            # D = H[k+1]-H[k]: tile 0 on DVE (it gates the whole mul chain);
            # tiles 1-3 on GPSIMD, interleaved with the last-block tails so
            # every D lands well before ACT needs it.
            nc.vector.tensor_sub(ds[0][:], hs[0][:, 2 : 2 * NP], hs[0][:, 0 : 2 * NSEG])

            def dsub(t):
                nc.gpsimd.tensor_sub(ds[t][:], hs[t][:, 2 : 2 * NP], hs[t][:, 0 : 2 * NSEG])

            def lastblock(t):
                """Last 8 subcarriers -> o[:, r, 511, :] (tiny GPSIMD ops)."""
                h = hs[t]
                h510 = h[:, 2 * NP - 4 : 2 * NP - 2].unsqueeze(1).broadcast_to((P, 8, 2))
                h511 = h[:, 2 * NP - 2 : 2 * NP].unsqueeze(1).broadcast_to((P, 8, 2))
                tl = lpool.tile([P, 8, 2], f16, name=f"tl{t}", tag="tl")
                nc.gpsimd.tensor_mul(tl[:], h510, a_last)
                t2 = lpool.tile([P, 8, 2], f16, name=f"t2{t}", tag="t2")
                nc.gpsimd.tensor_mul(t2[:], h511, c_last)
                nc.gpsimd.tensor_add(orvs[t][:, :, 2 * NSEG : 2 * NP], tl[:], t2[:])

            dsub(1)
            lastblock(0)
            dsub(2)
            lastblock(1)
            dsub(3)
            lastblock(2)
            lastblock(3)

            deferred = []
            W = 2 * NSEG
            for t in range(N_BT):
                h, o, orv, d = hs[t], os_[t], orvs[t], ds[t]
                if t in (1, 2):
                    # Paired-add experiment: one tmp tile holds rows m=0..4;
                    # each DVE op covers two r-phases with long dense runs.
                    TM = tpool.tile([P, 5, W], f16, name=f"tm{t}", tag="tmp5")
                    for m in range(4):
                        nc.scalar.mul(TM[:, m, :], d[:], ct[:, m : m + 1])
                    hN = h[:, 0:W].unsqueeze(1).broadcast_to((P, 2, W))
                    hP = h[:, 2 : 2 * NP].unsqueeze(1).broadcast_to((P, 2, W))
                    # pair (0,1): out = tmp + H_k
                    nc.vector.tensor_add(orv[:, 0:2, 0:W], TM[:, 0:2, :], hN)
                    # pair (7,6): out = H_{k+1} - tmp[1:3] (dst rows reversed)
                    nc.vector.tensor_sub(orv[:, 7:5:-1, 0:W], hP, TM[:, 1:3, :])
                    # pair (2,3)
                    nc.vector.tensor_add(orv[:, 2:4, 0:W], TM[:, 2:4, :], hN)
                    # single r5 = H_{k+1} - tmp3
                    nc.vector.tensor_sub(orv[:, 5, 0:W], h[:, 2 : 2 * NP], TM[:, 3, :])
                    nc.vector.tensor_scalar_mul(TM[:, 4, :], d[:], ct[:, 4:5])
                    nc.vector.tensor_add(orv[:, 4, 0:W], TM[:, 4, :], h[:, 0:W])
                else:
                    # 5 muls (tmp_r = gamma_r * D) + 8 dense adds/subs.
                    tmps = {}
                    for r in ADD_ORDER:
                        m = min(r, SPACING - r)
                        if m not in tmps:
                            tmp = tpool.tile([P, W], f16, name=f"tmp{t}_{m}", tag="tmp")
                            if (t, m) in DVE_MULS:
                                nc.vector.tensor_scalar_mul(tmp[:], d[:], ct[:, m : m + 1])
                            else:
                                nc.scalar.mul(tmp[:], d[:], ct[:, m : m + 1])
                            tmps[m] = tmp
                        dst = orv[:, r, 0:W]
                        if r <= 4:
                            nc.vector.tensor_add(dst, tmps[m][:], h[:, 0:W])
                        else:
                            nc.vector.tensor_sub(dst, h[:, 2 : 2 * NP], tmps[m][:])

                # Stores spread over three independent DMA rings: SP takes
                # t0/t1; t2 via gpsimd SWDGE, EMITTED after the loop (so its
                # sem-waits don't block Q7 compute); t3 rides ACT's ring,
                # whose configs sit after the last ACT mul.
                for r0, r1 in SGROUPS[t]:
                    if t == 2:
                        deferred.append((t, r0, r1, o))
                        continue
                    eng = nc.scalar if t == 3 and (r0, r1) in T3_ACT else nc.sync
                    eng.dma_start(
                        out[t * P : (t + 1) * P, r0 * RW : r1 * RW],
                        o[:, r0 * RW : r1 * RW],
                    )

            for t, r0, r1, o in deferred:
                nc.gpsimd.dma_start(
                    out[t * P : (t + 1) * P, r0 * RW : r1 * RW],
                    o[:, r0 * RW : r1 * RW],
                )
    nc.compile()
    return nc


def _coefs(decay_param: np.ndarray):
    """gamma [128,8] f32; last-block coefs on H510/H511 [128,32] f16."""
    x = np.float32(np.asarray(decay_param).reshape(-1)[0])
    d = np.logaddexp(np.float32(0.0), x, dtype=np.float32)  # softplus
    r = np.arange(SPACING, dtype=np.float32)
    eps = np.float32(1e-12)
    wl = np.exp(-d * r, dtype=np.float32)
    wr = np.exp(-d * (np.float32(SPACING) - r), dtype=np.float32)
    gamma = wr / (wl + wr + eps)
    # last block: i = 4088 + r, x0 = 4088, x1 = 4095 (gap of 7);
    # y1 = hN = 1.875*H[511] - 0.875*H[510]
    wl2 = np.exp(-d * r, dtype=np.float32)
    wr2 = np.exp(-d * (np.float32(7.0) - r), dtype=np.float32)
    w2 = wl2 + wr2 + eps
    c511 = (wl2 + np.float32(1.875) * wr2) / w2
    c510 = -np.float32(0.875) * wr2 / w2
    cg = np.broadcast_to(gamma, (P, 8)).astype(np.float32).copy()
    row = np.concatenate([np.repeat(c510, 2), np.repeat(c511, 2)])
    cl = np.broadcast_to(row, (P, 32)).astype(np.float16).copy()
    return cg, cl


def kernel(LS_ri, pilot_pos=None, decay_param=None, Nfft=None, **_unused):
    global _PROGRAM
    ls16 = np.ascontiguousarray(
        np.asarray(LS_ri, dtype=np.float32).reshape(B, NP * 2).astype(np.float16)
    )
    cg, cl = _coefs(decay_param)

    if _PROGRAM is None:
        _PROGRAM = _build_program()
    nc = _PROGRAM

    in_maps = []
    for c in range(N_CORES):
        in_maps.append(
            {"ls": ls16[c * B_LOC : (c + 1) * B_LOC], "cg": cg, "cl": cl}
        )

    res = run_bass_kernel_spmd(nc, in_maps, list(range(N_CORES))).results
    # device output is r-major [b, r, k, c]; de-interleave to [b, 8k+r, c]
    out = np.concatenate(
        [
            res[c]["out"]
            .reshape(B_LOC, SPACING, NP, 2)
            .transpose(0, 2, 1, 3)
            .reshape(B_LOC, NFFT, 2)
            for c in range(N_CORES)
        ],
        axis=0,
    ).astype(np.float32)
    return out


# revision 13
# speedup vs baseline: 1.0777x; 1.0777x over previous
"""Distance-aware comb-pilot interpolator for Trainium2 (8 NeuronCores).

Math: out[b, 8k+r, c] = alpha[r]*H[b,k,c] + gamma[r]*H[b,k+1,c], pilots on the
comb loc[k] = 8k (k = 0..511), Nfft = 4096.  Two identities cut the work:

  alpha[r] + gamma[r] = 1  (up to the reference's 1e-12 eps)
      -> out_r = H_k + gamma[r]*D_k  with  D = H[k+1] - H[k]
  alpha[8-r] = gamma[r]  (weight symmetry of the exp-decay kernel)
      -> out_{8-r} = H_{k+1} - gamma[r]*D_k   (reuses the same product)

So per batch-tile only 5 tensor_scalar muls (r = 0..4, on ACT mostly) and 8
dense tensor add/sub ops (DVE 2x fp16 mode) produce all 4096 subcarriers.

The device computes fp16 end-to-end and writes the output in r-major order
out_dev[b, r, k, c] so every DVE op touches only unit-stride APs (strided
dst was measured to drop DVE to 1x mode); the host de-interleaves with a
numpy transpose and upcasts to fp32 (rel err ~8e-4, gate is 2e-2).

HBM traffic per core: 1.05 MB in + 8.39 MB out ~= 26.4 us at 358 GB/s.
Engine busy: DVE ~24 us, ACT ~20 us, GPSIMD ~13 us (loads h2/h3, D-subs for
tiles 1-3, the 16-col last-block tail) -- memory-bound as intended.

The last 8 subcarriers (i = 4088..4095) interpolate between pilot 511 at
4088 and a virtual pilot hN = 1.875*H[511] - 0.875*H[510] at 4095 (gap 7,
not 8): per-r coefficients on H[510]/H[511], three tiny broadcast GPSIMD
ops per tile writing o[:, r, 511, :].
"""

import sys

import numpy as np

for _p in ("/opt/trn_rl_repo", "/root/.axon_site/_ro/trn_rl_repo"):
    if _p not in sys.path:
        sys.path.append(_p)

import concourse.bass as bass
import concourse.tile as tile
from concourse import bacc, mybir
from concourse.bass_utils import run_bass_kernel_spmd

N_CORES = 8
B, NP, NFFT, SPACING = 4096, 512, 4096, 8
B_LOC = B // N_CORES  # batch rows per core
NSEG = NP - 1  # regular 8-wide segments (k = 0..510)
P = 128  # SBUF partitions
N_BT = B_LOC // P  # 128-batch tiles per core
RW = 2 * NP  # o-tile columns per r-phase (512 k-slots x 2)

_PROGRAM = None


def _build_program():
    """One Bass program, identical on all cores (pure data parallel)."""
    nc = bacc.Bacc("TRN2", target_bir_lowering=False, debug=False)
    f16 = mybir.dt.float16
    f32 = mybir.dt.float32
    ls = nc.dram_tensor("ls", [B_LOC, NP * 2], f16, kind="ExternalInput").ap()
    cg = nc.dram_tensor("cg", [P, 8], f32, kind="ExternalInput").ap()
    cl = nc.dram_tensor("cl", [P, 32], f16, kind="ExternalInput").ap()
    out = nc.dram_tensor("out", [B_LOC, NFFT * 2], f16, kind="ExternalOutput").ap()

    # Mul engine per (tile, r): 'V' entries run on DVE (tensor_scalar, 4x
    # mode, no cross-engine latency -- used at the ramp (t0) and the drain
    # (t3 r4)); everything else on ACT.  Adds are issued right after the mul
    # they consume; the r-major store groups are ordered by completion.
    DVE_MULS = {(0, 0), (0, 4), (1, 4), (2, 4), (3, 4)}
    ADD_ORDER = [0, 1, 7, 2, 6, 3, 5, 4]  # add_r after mul_min(r,8-r)
    SGROUPS = {
        0: [(0, 1), (1, 2), (6, 8), (2, 4), (4, 6)],
        1: [(0, 2), (6, 8), (2, 4), (4, 6)],
        2: [(0, 2), (6, 8), (2, 4), (4, 6)],
        3: [(0, 1), (1, 2), (7, 8), (2, 3), (6, 7), (3, 4), (5, 6), (4, 5)],
    }
    # t3's trailing 4 store configs ride ACT's ring (they sit after the last
    # ACT mul in its queue, so their sem-waits block nothing), keeping the
    # final drain to one 256 KB chunk.
    T3_ACT = {(6, 7), (3, 4), (5, 6), (4, 5)}

    with tile.TileContext(nc) as tc:
        with (
            tc.tile_pool(name="cpool", bufs=1) as cpool,
            tc.tile_pool(name="hpool", bufs=4) as hpool,
            tc.tile_pool(name="dpool", bufs=4) as dpool,
            tc.tile_pool(name="tpool", bufs=10) as tpool,
            tc.tile_pool(name="opool", bufs=4) as opool,
            tc.tile_pool(name="lpool", bufs=8) as lpool,
        ):
            # Loads: h0 first on ACT's HWDGE ring (fast first byte, gates the
            # whole pipeline), the tiny gamma tile right after; ctl + h1 on
            # SP's HWDGE (idle until the first store); h2/h3 via gpsimd SWDGE.
            hs = [hpool.tile([P, NP * 2], f16, name=f"h{t}", tag="h") for t in range(N_BT)]
            ct = cpool.tile([P, 8], f32)
            nc.sync.dma_start(ct[:], cg)
            nc.scalar.dma_start(hs[0][:], ls[0:P, :])
            ctl = cpool.tile([P, 32], f16)
            nc.sync.dma_start(ctl[:], cl)
            nc.sync.dma_start(hs[1][:], ls[P : 2 * P, :])
            # h2/h3 queue behind h0 on ACT's ring: h0 keeps HBM priority
            # during the ramp, Q7 stays free for the D subs.
            nc.scalar.dma_start(hs[2][:], ls[2 * P : 3 * P, :])
            nc.scalar.dma_start(hs[3][:], ls[3 * P : 4 * P, :])

            a_last = ctl[:, 0:16].rearrange("p (r c) -> p r c", c=2)
            c_last = ctl[:, 16:32].rearrange("p (r c) -> p r c", c=2)

            os_ = [opool.tile([P, NFFT * 2], f16, name=f"o{t}", tag="o") for t in range(N_BT)]
            orvs = [o[:].rearrange("p (r k) -> p r k", r=SPACING) for o in os_]
            ds = [dpool.tile([P, 2 * NSEG], f16, name=f"d{t}", tag="d") for t in range(N_BT)]

            # D = H[k+1]-H[k]: tile 0 on DVE (it gates the whole mul chain);
            # tiles 1-3 on GPSIMD, interleaved with the last-block tails so
            # every D lands well before ACT needs it.
            nc.vector.tensor_sub(ds[0][:], hs[0][:, 2 : 2 * NP], hs[0][:, 0 : 2 * NSEG])

            def dsub(t):
                nc.gpsimd.tensor_sub(ds[t][:], hs[t][:, 2 : 2 * NP], hs[t][:, 0 : 2 * NSEG])

            def lastblock(t):
                """Last 8 subcarriers -> o[:, r, 511, :] (tiny GPSIMD ops)."""
                h = hs[t]
                h510 = h[:, 2 * NP - 4 : 2 * NP - 2].unsqueeze(1).broadcast_to((P, 8, 2))
                h511 = h[:, 2 * NP - 2 : 2 * NP].unsqueeze(1).broadcast_to((P, 8, 2))
                tl = lpool.tile([P, 8, 2], f16, name=f"tl{t}", tag="tl")
                nc.gpsimd.tensor_mul(tl[:], h510, a_last)
                t2 = lpool.tile([P, 8, 2], f16, name=f"t2{t}", tag="t2")
                nc.gpsimd.tensor_mul(t2[:], h511, c_last)
                nc.gpsimd.tensor_add(orvs[t][:, :, 2 * NSEG : 2 * NP], tl[:], t2[:])

            dsub(1)
            lastblock(0)
            dsub(2)
            lastblock(1)
            dsub(3)
            lastblock(2)
            lastblock(3)

            deferred = []
            W = 2 * NSEG
            for t in range(N_BT):
                h, o, orv, d = hs[t], os_[t], orvs[t], ds[t]
                # One tmp tile holds rows m=0..4; paired adds cover two
                # r-phases per DVE op (long dense runs keep 2x mode).
                # Mul engines: m4 always DVE; t0 also m0/m1 on DVE (ramp).
                TM = tpool.tile([P, 5, W], f16, name=f"tm{t}", tag="tmp5")
                for m in range(4):
                    if t == 0 and m < 2:
                        nc.vector.tensor_scalar_mul(TM[:, m, :], d[:], ct[:, m : m + 1])
                    else:
                        nc.scalar.mul(TM[:, m, :], d[:], ct[:, m : m + 1])
                hN = h[:, 0:W].unsqueeze(1).broadcast_to((P, 2, W))
                hP = h[:, 2 : 2 * NP].unsqueeze(1).broadcast_to((P, 2, W))
                # pair (0,1): out = tmp + H_k
                nc.vector.tensor_add(orv[:, 0:2, 0:W], TM[:, 0:2, :], hN)
                # pair (7,6): out = H_{k+1} - tmp[1:3] (dst rows reversed)
                nc.vector.tensor_sub(orv[:, 7:5:-1, 0:W], hP, TM[:, 1:3, :])
                # pair (2,3)
                nc.vector.tensor_add(orv[:, 2:4, 0:W], TM[:, 2:4, :], hN)
                # singles: r5 = H_{k+1} - tmp3; r4 = tmp4 + H_k
                nc.vector.tensor_sub(orv[:, 5, 0:W], h[:, 2 : 2 * NP], TM[:, 3, :])
                nc.vector.tensor_scalar_mul(TM[:, 4, :], d[:], ct[:, 4:5])
                nc.vector.tensor_add(orv[:, 4, 0:W], TM[:, 4, :], h[:, 0:W])

                # Stores spread over three independent DMA rings: SP takes
                # t0/t1; t2 via gpsimd SWDGE, EMITTED after the loop (so its
                # sem-waits don't block Q7 compute); t3 rides ACT's ring,
                # whose configs sit after the last ACT mul.
                for r0, r1 in SGROUPS[t]:
                    if t == 2:
                        deferred.append((t, r0, r1, o))
                        continue
                    eng = nc.scalar if t == 3 and (r0, r1) in T3_ACT else nc.sync
                    eng.dma_start(
                        out[t * P : (t + 1) * P, r0 * RW : r1 * RW],
                        o[:, r0 * RW : r1 * RW],
                    )

            for t, r0, r1, o in deferred:
                nc.gpsimd.dma_start(
                    out[t * P : (t + 1) * P, r0 * RW : r1 * RW],
                    o[:, r0 * RW : r1 * RW],
                )
    nc.compile()
    return nc


def _coefs(decay_param: np.ndarray):
    """gamma [128,8] f32; last-block coefs on H510/H511 [128,32] f16."""
    x = np.float32(np.asarray(decay_param).reshape(-1)[0])
    d = np.logaddexp(np.float32(0.0), x, dtype=np.float32)  # softplus
    r = np.arange(SPACING, dtype=np.float32)
    eps = np.float32(1e-12)
    wl = np.exp(-d * r, dtype=np.float32)
    wr = np.exp(-d * (np.float32(SPACING) - r), dtype=np.float32)
    gamma = wr / (wl + wr + eps)
    # last block: i = 4088 + r, x0 = 4088, x1 = 4095 (gap of 7);
    # y1 = hN = 1.875*H[511] - 0.875*H[510]
    wl2 = np.exp(-d * r, dtype=np.float32)
    wr2 = np.exp(-d * (np.float32(7.0) - r), dtype=np.float32)
    w2 = wl2 + wr2 + eps
    c511 = (wl2 + np.float32(1.875) * wr2) / w2
    c510 = -np.float32(0.875) * wr2 / w2
    cg = np.broadcast_to(gamma, (P, 8)).astype(np.float32).copy()
    row = np.concatenate([np.repeat(c510, 2), np.repeat(c511, 2)])
    cl = np.broadcast_to(row, (P, 32)).astype(np.float16).copy()
    return cg, cl


def kernel(LS_ri, pilot_pos=None, decay_param=None, Nfft=None, **_unused):
    global _PROGRAM
    ls16 = np.ascontiguousarray(
        np.asarray(LS_ri, dtype=np.float32).reshape(B, NP * 2).astype(np.float16)
    )
    cg, cl = _coefs(decay_param)

    if _PROGRAM is None:
        _PROGRAM = _build_program()
    nc = _PROGRAM

    in_maps = []
    for c in range(N_CORES):
        in_maps.append(
            {"ls": ls16[c * B_LOC : (c + 1) * B_LOC], "cg": cg, "cl": cl}
        )

    res = run_bass_kernel_spmd(nc, in_maps, list(range(N_CORES))).results
    # device output is r-major [b, r, k, c]; de-interleave to [b, 8k+r, c]
    out = np.concatenate(
        [
            res[c]["out"]
            .reshape(B_LOC, SPACING, NP, 2)
            .transpose(0, 2, 1, 3)
            .reshape(B_LOC, NFFT, 2)
            for c in range(N_CORES)
        ],
        axis=0,
    ).astype(np.float32)
    return out
